# revision 16
# baseline (speedup 1.0000x reference)
"""ColBERT intra-batch MaxSim scoring kernel for 8 Trainium2 NeuronCores.

Math (see reference):
  Q = l2norm(q_hidden @ W.T)                       [B, LQ, DIM]
  D = l2norm(d_hidden @ W.T); D masked             [B, LD, DIM]
  sim[b,c,q,k] = Q[b,q]·D[c,k]; masked k -> -inf
  out[b,c] = sum_q max_k sim

Sharding: docs (dim c) are sharded 16-per-core; q_hidden/W replicated.
Each core computes its [B, 16] slice of the score matrix.

Device-side structure (v2 — "LIN1" max-split):
  * Host pre-transposes activations to [HID, tokens] (bf16 to halve HBM
    traffic) and folds the doc mask away by gathering valid tokens to the
    front, padding the tail with copies of the doc's first valid token
    (duplicates never change a max).
  * The max over each doc's NV tokens is split one level in the *linear*
    domain: with per-doc token halves h0/h1 and the normalized doc
    embeddings D0/D1, max(s0,s1) = s0 + relu(s1-s0), and s1-s0 is itself
    a matmul against Ddelta = D1-D0.  So per 8-doc group the device runs
      PE:  S0 = Q·D0 (PSUM, group open)     SΔ = Q·Ddelta (PSUM)
      ACT: r = relu(SΔ) -> SBUF bf16
      PE:  S0 += I·r  (identity matmul accumulates into the open group)
      DVE: reduce_max over NV/2-wide doc slots
    which halves the DVE tensor_reduce (the v1 bottleneck at ~90us busy)
    at the cost of ~1.5x PE rows and the ACT relu, balancing all three
    engines.
  * Q is NOT normalized before the sim matmuls: max_k is invariant under
    a positive per-query scale, so 1/|Q| is folded into the block-ones
    lhsT of the final query-sum matmul.  Per-token |Q|^2 is computed by
    ones-matmuls into spare columns of the Q-projection PSUM bank.
  * D norms: ones-matmul sumsq row, sqrt (ACT) + fast reciprocal (DVE),
    broadcast via K=1 ones outer-product, applied by the DVE while
    splitting D into D0 / D1 / Ddelta.
  * PSUM budget (8 banks): psq 1 | S0 double-buffered 2x2 | SΔ 2 |
    psout 1.
"""

import os

import numpy as np

B, LQ, LD, HID, DIM = 128, 32, 256, 768, 128
NCORES = 8
DPC = B // NCORES          # docs per core
TQ = B * LQ                # total query tokens
KC = HID // 128            # contraction chunks for the projection
GD = 8                     # docs per sim group (2 groups per 128-token tile)

SIM_MODE = os.environ.get("KERNEL_SIM_MODE", "bf16")


def _chunks(total, step):
    """[(off, len)] cut at `step` boundaries — a matmul's PSUM output must
    stay inside a single 512-float bank, so chunks may never straddle one."""
    return [(o, min(step, total - o)) for o in range(0, total, step)]


def _build_program(NV):
    import concourse.bass as bass  # noqa: F401
    import concourse.tile as tile
    from concourse import bacc, mybir

    f32 = mybir.dt.float32
    AF = mybir.ActivationFunctionType
    AX = mybir.AxisListType
    ALU = mybir.AluOpType

    sim_dt = {"bf16": mybir.dt.bfloat16, "f32": f32}[SIM_MODE]
    proj_dt = sim_dt

    NV2 = NV // 2           # compacted tokens per doc half
    NVT = DPC * NV          # compacted doc tokens per core
    NM = DPC * NV2          # columns after the LIN1 fold (also = |Ddelta|)
    GW = GD * NV2           # sim-group width (8 docs)
    NG = DPC // GD          # sim groups per tile
    NQCH = TQ // 512        # q-projection column chunks
    NTT = TQ // 128         # sim lhsT tiles (query-token tiles)
    BPT = 128 // LQ         # batch entries per query-token tile
    QG = 1024               # qt DMA column-group width
    d_chunks = _chunks(NVT, 512)   # d-projection column chunks
    g_chunks = _chunks(GW, 512)    # sim-group matmul N chunks (512 + 96)

    nc = bacc.Bacc(
        "TRN2",
        target_bir_lowering=False,
        debug=False,
        num_devices=NCORES,
    )

    qT_d = nc.dram_tensor("qT", [HID, TQ], proj_dt, kind="ExternalInput")
    dT_d = nc.dram_tensor("dT", [HID, NVT], proj_dt, kind="ExternalInput")
    wT_d = nc.dram_tensor("wT", [128, KC, DIM], proj_dt, kind="ExternalInput")
    qso_d = nc.dram_tensor("qso", [128, BPT], sim_dt, kind="ExternalInput")
    onescol_d = nc.dram_tensor("onescol", [128, 1], sim_dt, kind="ExternalInput")
    onesrow_d = nc.dram_tensor("onesrow", [1, 128], sim_dt, kind="ExternalInput")
    ident_d = nc.dram_tensor("ident", [128, 128], sim_dt, kind="ExternalInput")
    out_d = nc.dram_tensor("out", [B, DPC], f32, kind="ExternalOutput")
    dbg = os.environ.get("KERNEL_DEBUG_OUT", "0") == "1"
    if dbg:
        dbgQT_d = nc.dram_tensor("dbg_QT", [128, TQ], sim_dt, kind="ExternalOutput")
        dbgD0_d = nc.dram_tensor("dbg_D0", [128, NM], sim_dt, kind="ExternalOutput")
        dbgDD_d = nc.dram_tensor("dbg_DD", [128, NM], sim_dt, kind="ExternalOutput")
        dbgMall_d = nc.dram_tensor("dbg_mall", [128, NTT, DPC], f32, kind="ExternalOutput")

    with tile.TileContext(nc) as tc, tc.tile_pool(name="persist", bufs=1) as per:
        # --- constants + persistent SBUF tensors ---------------------------
        wt = per.tile([128, KC, DIM], proj_dt, name="wt")
        qso = per.tile([128, BPT], sim_dt, name="qso")
        onescol = per.tile([128, 1], sim_dt, name="onescol")
        onesrow = per.tile([1, 128], sim_dt, name="onesrow")
        ident = per.tile([128, 128], sim_dt, name="ident")
        QT = per.tile([128, TQ], sim_dt, name="QT")       # q-proj [d, t] unnormalized
        DTn0 = per.tile([128, NM], sim_dt, name="DTn0")   # normalized doc half 0
        DTnH = per.tile([128, NM], sim_dt, name="DTnH")   # normalized doc half 1
        DTnD = per.tile([128, NM], sim_dt, name="DTnD")   # half1 - half0
        invnQ = per.tile([128, NTT], f32, name="invnQ")   # 1/|Q| per query token
        normQ = per.tile([128, NTT], f32, name="normQ")
        ssqD_row = per.tile([1, NVT], sim_dt, name="ssqD_row")
        bc = per.tile([128, NVT], f32, name="bc")         # broadcast 1/|D|
        outstage = per.tile([BPT, NTT * DPC], f32, name="outstage")
        if dbg:
            dbgM = per.tile([128, NTT, DPC], f32, name="dbgM")

        nc.sync.dma_start(wt[:], wT_d[:, :, :])
        nc.scalar.dma_start(ident[:], ident_d[:, :])
        nc.gpsimd.dma_start(qso[:], qso_d[:, :])
        nc.gpsimd.dma_start(onescol[:], onescol_d[:, :])
        nc.gpsimd.dma_start(onesrow[:], onesrow_d[:, :])

        # psq pool is opened before the D-phase pools and stays open across
        # the whole kernel: bank 0 is the Q-projection bank in both phases.
        psq_stack = tc.tile_pool(name="psqp", bufs=1, space="PSUM")
        psq_pool = psq_stack.__enter__()
        qs_stack = tc.tile_pool(name="qt_pool", bufs=1)
        qt_pool = qs_stack.__enter__()
        sqq_stack = tc.tile_pool(name="sqQ_pool", bufs=2)
        sqQ_pool = sqq_stack.__enter__()

        qts = {}

        def load_jg(jg):
            for k in range(KC):
                t_ = qt_pool.tile(
                    [128, QG], proj_dt, name=f"qt{k}_{jg}", tag=f"qt{k}",
                    bufs=3,
                )
                eng = nc.sync if k % 2 == 0 else nc.scalar
                eng.dma_start(t_[:], qT_d[k * 128:(k + 1) * 128,
                                          jg * QG:(jg + 1) * QG])
                qts[(k, jg)] = t_

        psq_live = {}

        def project_mm(j, ks):
            jg, r = divmod(j * 512, QG)
            if j not in psq_live:
                psq_live[j] = psq_pool.tile([128, 512], f32, name=f"psq{j}",
                                            tag="psq")
            psq = psq_live[j]
            for k in ks:
                nc.tensor.matmul(
                    psq[:, 0:512],
                    wt[:, k, :],
                    qts[(k, jg)][:, r:r + 512],
                    start=(k == 0),
                    stop=(k == KC - 1),
                )

        def project(j):
            sl = slice(j * 512, (j + 1) * 512)
            psq = psq_live.pop(j)
            nc.vector.tensor_copy(QT[:, sl], psq[:, 0:512])
            sq = sqQ_pool.tile([128, 512], sim_dt, name="sqq", tag="sqq")
            nc.scalar.activation(sq[:], psq[:, 0:512], AF.Square)
            # per-token |Q|^2 via ones-matmuls into spare psq columns (the
            # proj values there were already consumed by the two reads)
            for s in range(4):
                nc.tensor.matmul(
                    psq[:, 508 + s:509 + s],
                    sq[:, s * 128:(s + 1) * 128],
                    onescol[:],
                    start=True,
                    stop=True,
                )
            csl = slice(j * 4, (j + 1) * 4)
            nc.scalar.activation(normQ[:, csl], psq[:, 508:512], AF.Sqrt)
            nc.vector.reciprocal(invnQ[:, csl], normQ[:, csl])

        # PE warm-up: the HAM clock gate needs ~3.4us of sustained activity
        # to lift the PE from 1.2 to 2.4 GHz; burn the initial dT DMA wait
        # on dummy matmuls so the D projection starts warm.
        warm = psq_pool.tile([128, 128], f32, name="warm", tag="psq")
        for _ in range(32):
            nc.tensor.matmul(warm[:], ident[:], ident[:], start=True, stop=True)

        # ---------------- phase D: project + normalize doc tokens ----------
        with (
            tc.tile_pool(name="dt_pool", bufs=1) as dt_pool,
            tc.tile_pool(name="psD", bufs=1, space="PSUM") as psD,
            tc.tile_pool(name="ssD", bufs=1, space="PSUM") as ssD,
            tc.tile_pool(name="sqD_pool", bufs=2) as sqD_pool,
            tc.tile_pool(name="psB", bufs=1, space="PSUM") as psB,
        ):
            dts = []
            for k in range(KC):
                dtk = dt_pool.tile([128, NVT], proj_dt, name=f"dt{k}", tag=f"dt{k}")
                dts.append(dtk)
            # column-halved transfers, all first halves ahead of all second
            # halves: the k-outer accumulation over cols [0, NM) completes as
            # soon as the 6 a-halves land, so the norm chain starts earlier
            for h in range(2):
                cs = slice(h * NM, NVT if h else NM)
                for k in range(KC):
                    eng = nc.sync if k % 2 == 0 else nc.scalar
                    eng.dma_start(dts[k][:, cs], dT_d[k * 128:(k + 1) * 128, cs])
            load_jg(0)
            load_jg(1)

            psd = psD.tile([128, NVT], f32, name="psd")
            for k in range(KC):
                for (off, ln) in d_chunks:
                    nc.tensor.matmul(
                        psd[:, off:off + ln],
                        wt[:, k, :],
                        dts[k][:, off:off + ln],
                        start=(k == 0),
                        stop=(k == KC - 1),
                    )
            for (off, ln) in d_chunks:
                sl = slice(off, off + ln)
                sq = sqD_pool.tile([128, 512], sim_dt, name="sqd", tag="sq")
                nc.scalar.activation(sq[:, :ln], psd[:, sl], AF.Square)
                ssd = ssD.tile([1, 512], f32, name="ssd", tag="ssd")
                nc.tensor.matmul(
                    ssd[:, :ln], onescol[:], sq[:, :ln], start=True, stop=True
                )
                nc.scalar.copy(ssqD_row[:, sl], ssd[:, :ln])

            # broadcast |D|^2 across partitions (bf16 ones-outer-product),
            # then full-width sqrt (ACT) + ~51-ULP reciprocal (DVE) per chunk
            for (off, ln) in d_chunks:
                sl = slice(off, off + ln)
                psb = psB.tile([128, 512], f32, name="psb", tag="psb")
                nc.tensor.matmul(
                    psb[:, :ln], onesrow[:], ssqD_row[:, sl], start=True, stop=True
                )
                rsq = sqD_pool.tile([128, 512], f32, name="rsq", tag="rsq")
                nc.scalar.activation(rsq[:, :ln], psb[:, :ln], AF.Sqrt)
                nc.vector.reciprocal_approx_fast(bc[:, sl], rsq[:, :ln])

            # normalize + split halves + delta (DVE; psd stays live here),
            # emitted per 8-doc half so group-0 sims can start after half 0
            psd_v = psd[:].rearrange("p (d v) -> p d v", v=NV)
            bc_v = bc[:].rearrange("p (d v) -> p d v", v=NV)
            D0_v = DTn0[:].rearrange("p (d v) -> p d v", v=NV2)
            DH_v = DTnH[:].rearrange("p (d v) -> p d v", v=NV2)
            for h in range(2):
                ds = slice(h * GD, (h + 1) * GD)
                fs = slice(h * GW, (h + 1) * GW)
                nc.vector.tensor_tensor(
                    D0_v[:, ds], psd_v[:, ds, 0:NV2], bc_v[:, ds, 0:NV2],
                    op=ALU.mult,
                )
                nc.vector.tensor_tensor(
                    DH_v[:, ds], psd_v[:, ds, NV2:NV], bc_v[:, ds, NV2:NV],
                    op=ALU.mult,
                )
                nc.vector.tensor_tensor(
                    DTnD[:, fs], DTnH[:, fs], DTn0[:, fs], op=ALU.subtract
                )

            # early Q chunks overlap the D-phase tail
            project_mm(0, range(KC))
            project(0)
            project_mm(1, range(KC))
            project(1)
            load_jg(2)

        # ---------- phase S: sim groups with LIN1 fold ---------------------
        # One merged 3-bank PSUM tile per 8-doc group, laid out so the BIG
        # relu waits only on its own delta matmul (bank 1 is pure delta):
        #   bank 0: M for docs 0..5 (W1=6*NV2)   bank 1: delta docs 0..5
        #   bank 2: M docs 6..7 (W2) + delta docs 6..7
        # bank-2 order delta->M (each start=True clears the whole bank's
        # has_written bits; M must clear last so the identity matmul
        # accumulates onto M only).
        W1 = 6 * NV2
        W2 = 2 * NV2
        assert W1 <= 512 and 2 * W2 <= 512

        with (
            tc.tile_pool(name="psS", bufs=2, space="PSUM") as psS,
            tc.tile_pool(name="psO", bufs=1, space="PSUM") as psO,
            tc.tile_pool(name="r_pool", bufs=2) as r_pool,
            tc.tile_pool(name="m_pool", bufs=2) as m_pool,
        ):
            psout = psO.tile([BPT, NTT * DPC], f32, name="psout")

            def simgroup(t, g, mall, weave=None):
                lq = QT[:, t * 128:(t + 1) * 128]
                d0 = g * GW
                ps = psS.tile([128, 1536], f32, name="ps", tag="ps")
                nc.tensor.matmul(                       # delta docs 0..5
                    ps[:, 512:512 + W1], lq, DTnD[:, d0:d0 + W1],
                    start=True, stop=True,
                )
                nc.tensor.matmul(                       # delta docs 6..7
                    ps[:, 1024 + W2:1024 + 2 * W2], lq,
                    DTnD[:, d0 + W1:d0 + GW], start=True, stop=True,
                )
                nc.tensor.matmul(                       # M docs 6..7
                    ps[:, 1024:1024 + W2], lq, DTn0[:, d0 + W1:d0 + GW],
                    start=True, stop=False,
                )
                nc.tensor.matmul(                       # M docs 0..5
                    ps[:, 0:W1], lq, DTn0[:, d0:d0 + W1],
                    start=True, stop=False,
                )
                r = r_pool.tile([128, GW], sim_dt, name="r", tag="r")
                nc.scalar.activation(r[:, 0:W1], ps[:, 512:512 + W1], AF.Relu)
                nc.scalar.activation(
                    r[:, W1:GW], ps[:, 1024 + W2:1024 + 2 * W2], AF.Relu
                )
                if weave:
                    weave()
                nc.tensor.matmul(
                    ps[:, 0:W1], ident[:], r[:, 0:W1], start=False, stop=True,
                )
                nc.tensor.matmul(
                    ps[:, 1024:1024 + W2], ident[:], r[:, W1:GW],
                    start=False, stop=True,
                )
                nc.vector.reduce_max(
                    mall[:, g * GD:g * GD + 6],
                    ps[:, 0:W1].rearrange("p (d v) -> p d v", v=NV2),
                    axis=AX.X,
                )
                nc.vector.reduce_max(
                    mall[:, g * GD + 6:(g + 1) * GD],
                    ps[:, 1024:1024 + W2].rearrange("p (d v) -> p d v", v=NV2),
                    axis=AX.X,
                )

            mals_q = []

            def flush_psout():
                t_, mals_ = mals_q.pop(0)
                nc.tensor.matmul(
                    psout[:, t_ * DPC:(t_ + 1) * DPC],
                    qso[:],
                    mals_[:],
                    start=True,
                    stop=True,
                )

            def simtile(t, weave=None):
                # the psout matmul of tile t-2 goes first: its gpsimd-scaled
                # rhs is ready by now, so it never head-of-line-blocks the
                # strict-FIFO PE queue the way an eagerly-issued one would
                if len(mals_q) >= 2:
                    flush_psout()
                mall = m_pool.tile([128, DPC], f32, name="mall", tag="mall",
                                   bufs=4)
                for g in range(NG):
                    simgroup(t, g, mall, weave)
                # fold 1/|Q| into the maxes on the (otherwise idle) gpsimd,
                # so the final sum matmul uses the constant block-ones qso
                mals = m_pool.tile([128, DPC], sim_dt, name="mals", tag="mals",
                                   bufs=4)
                nc.gpsimd.tensor_scalar(
                    mals[:], mall[:], invnQ[:, t:t + 1], None, op0=ALU.mult
                )
                if dbg:
                    nc.vector.tensor_copy(dbgM[:, t, :], mals[:])
                mals_q.append((t, mals))

            for j in range(NQCH):
                kstep = iter(range(KC))

                def weave(_j=j, _ks=kstep):
                    if _j + 2 < NQCH:
                        k = next(_ks, None)
                        if k is not None:
                            project_mm(_j + 2, [k])

                if j == 1:
                    load_jg(3)
                for ti, t in enumerate(range(j * 4, (j + 1) * 4)):
                    simtile(t, weave=weave)
                    if j + 2 < NQCH and ti == 2:
                        project(j + 2)
            while mals_q:
                flush_psout()
            if dbg:
                nc.sync.dma_start(dbgQT_d[:, :], QT[:])
                nc.sync.dma_start(dbgD0_d[:, :], DTn0[:])
                nc.sync.dma_start(dbgDD_d[:, :], DTnD[:])
                nc.sync.dma_start(dbgMall_d[:, :, :], dbgM[:])
            nc.scalar.copy(outstage[:], psout[:])
            nc.sync.dma_start(
                out_d[:, :].rearrange("(t f) c -> f t c", f=BPT),
                outstage[:].rearrange("f (t c) -> f t c", c=DPC),
            )
        sqq_stack.__exit__(None, None, None)
        qs_stack.__exit__(None, None, None)
        psq_stack.__exit__(None, None, None)

    nc.compile()
    return nc


def _host_prep(q_hidden, d_hidden, W, d_mask):
    import ml_dtypes

    q = np.ascontiguousarray(np.asarray(q_hidden, dtype=np.float32))
    d = np.ascontiguousarray(np.asarray(d_hidden, dtype=np.float32))
    w = np.ascontiguousarray(np.asarray(W, dtype=np.float32))
    mask = np.asarray(d_mask, dtype=bool)

    nv = mask.sum(axis=1)
    NV = int(-(-max(int(nv.max()), 16) // 8) * 8)
    NV = min(NV, ((LD + 7) // 8) * 8)

    # per-doc gather indices: valid tokens first, padded with the first
    # valid token (duplicates never change a max)
    idx = np.zeros((B, NV), dtype=np.intp)
    for c in range(B):
        v = np.flatnonzero(mask[c])
        row = np.full(NV, v[0], dtype=np.intp)
        row[:min(len(v), NV)] = v[:NV]
        idx[c] = row

    dG = d[np.arange(B)[:, None], idx, :]          # [B, NV, HID]

    # activations/weights are shipped bf16 so HBM traffic halves
    proj_np = ml_dtypes.bfloat16 if SIM_MODE == "bf16" else np.float32
    qT = np.ascontiguousarray(q.reshape(TQ, HID).T.astype(proj_np))  # [HID, TQ]
    # W.T rearranged so the [128, KC, DIM] SBUF tile is one contiguous DMA:
    # wTp[p, k, d] = W[d, k*128+p]
    wT = np.ascontiguousarray(
        w.T.reshape(KC, 128, DIM).transpose(1, 0, 2).astype(proj_np)
    )
    dT_cores = []
    for m in range(NCORES):
        blk = dG[m * DPC:(m + 1) * DPC].reshape(DPC * NV, HID)
        dT_cores.append(np.ascontiguousarray(blk.T.astype(proj_np)))

    qso = np.zeros((128, 128 // LQ), dtype=ml_dtypes.bfloat16 if SIM_MODE == "bf16" else np.float32)
    for p in range(128):
        qso[p, p // LQ] = 1.0
    ones_dt = ml_dtypes.bfloat16 if SIM_MODE == "bf16" else np.float32
    onescol = np.ones((128, 1), dtype=ones_dt)
    onesrow = np.ones((1, 128), dtype=ones_dt)
    ident = np.eye(128, dtype=ones_dt)
    return NV, qT, wT, dT_cores, qso, onescol, onesrow, ident


def kernel(q_hidden, d_hidden, W, d_mask):
    from concourse.bass_utils import run_bass_kernel_spmd

    NV, qT, wT, dT_cores, qso, onescol, onesrow, ident = _host_prep(
        q_hidden, d_hidden, W, d_mask
    )
    nc = _build_program(NV)

    in_maps = [
        {
            "qT": qT,
            "dT": dT_cores[m],
            "wT": wT,
            "qso": qso,
            "onescol": onescol,
            "onesrow": onesrow,
            "ident": ident,
        }
        for m in range(NCORES)
    ]
    res = run_bass_kernel_spmd(nc, in_maps, core_ids=list(range(NCORES)))
    out = np.concatenate(
        [res.results[m]["out"] for m in range(NCORES)], axis=1
    )
    return np.ascontiguousarray(out.astype(np.float32))


# revision 17
# speedup vs baseline: 1.3192x; 1.3192x over previous
"""ColBERT intra-batch MaxSim scoring kernel for 8 Trainium2 NeuronCores.

Math (see reference):
  Q = l2norm(q_hidden @ W.T)                       [B, LQ, DIM]
  D = l2norm(d_hidden @ W.T); D masked             [B, LD, DIM]
  sim[b,c,q,k] = Q[b,q]·D[c,k]; masked k -> -inf
  out[b,c] = sum_q max_k sim

Sharding: docs (dim c) are sharded 16-per-core; q_hidden/W replicated.
Each core computes its [B, 16] slice of the score matrix.

Device-side structure (v2 — "LIN1" max-split):
  * Host pre-transposes activations to [HID, tokens] (bf16 to halve HBM
    traffic) and folds the doc mask away by gathering valid tokens to the
    front, padding the tail with copies of the doc's first valid token
    (duplicates never change a max).
  * The max over each doc's NV tokens is split one level in the *linear*
    domain: with per-doc token halves h0/h1 and the normalized doc
    embeddings D0/D1, max(s0,s1) = s0 + relu(s1-s0), and s1-s0 is itself
    a matmul against Ddelta = D1-D0.  So per 8-doc group the device runs
      PE:  S0 = Q·D0 (PSUM, group open)     SΔ = Q·Ddelta (PSUM)
      ACT: r = relu(SΔ) -> SBUF bf16
      PE:  S0 += I·r  (identity matmul accumulates into the open group)
      DVE: reduce_max over NV/2-wide doc slots
    which halves the DVE tensor_reduce (the v1 bottleneck at ~90us busy)
    at the cost of ~1.5x PE rows and the ACT relu, balancing all three
    engines.
  * Q is NOT normalized before the sim matmuls: max_k is invariant under
    a positive per-query scale, so 1/|Q| is folded into the block-ones
    lhsT of the final query-sum matmul.  Per-token |Q|^2 is computed by
    ones-matmuls into spare columns of the Q-projection PSUM bank.
  * D norms: ones-matmul sumsq row, sqrt (ACT) + fast reciprocal (DVE),
    broadcast via K=1 ones outer-product, applied by the DVE while
    splitting D into D0 / D1 / Ddelta.
  * PSUM budget (8 banks): psq 1 | S0 double-buffered 2x2 | SΔ 2 |
    psout 1.
"""

import os

import numpy as np

B, LQ, LD, HID, DIM = 128, 32, 256, 768, 128
NCORES = 8
DPC = B // NCORES          # docs per core
TQ = B * LQ                # total query tokens
KC = HID // 128            # contraction chunks for the projection
GD = 8                     # docs per sim group (2 groups per 128-token tile)

SIM_MODE = os.environ.get("KERNEL_SIM_MODE", "bf16")


def _chunks(total, step):
    """[(off, len)] cut at `step` boundaries — a matmul's PSUM output must
    stay inside a single 512-float bank, so chunks may never straddle one."""
    return [(o, min(step, total - o)) for o in range(0, total, step)]


def _build_program(NV):
    import concourse.bass as bass  # noqa: F401
    import concourse.tile as tile
    from concourse import bacc, mybir

    f32 = mybir.dt.float32
    AF = mybir.ActivationFunctionType
    AX = mybir.AxisListType
    ALU = mybir.AluOpType

    sim_dt = {"bf16": mybir.dt.bfloat16, "f32": f32}[SIM_MODE]
    proj_dt = sim_dt

    NV2 = NV // 2           # compacted tokens per doc half
    NVT = DPC * NV          # compacted doc tokens per core
    NM = DPC * NV2          # columns after the LIN1 fold (also = |Ddelta|)
    GW = GD * NV2           # sim-group width (8 docs)
    NG = DPC // GD          # sim groups per tile
    NQCH = TQ // 512        # q-projection column chunks
    NTT = TQ // 128         # sim lhsT tiles (query-token tiles)
    BPT = 128 // LQ         # batch entries per query-token tile
    QG = 1024               # qt DMA column-group width
    d_chunks = _chunks(NVT, 512)   # d-projection column chunks
    g_chunks = _chunks(GW, 512)    # sim-group matmul N chunks (512 + 96)

    nc = bacc.Bacc(
        "TRN2",
        target_bir_lowering=False,
        debug=False,
        num_devices=NCORES,
    )

    qT_d = nc.dram_tensor("qT", [HID, TQ], proj_dt, kind="ExternalInput")
    dT_d = nc.dram_tensor("dT", [HID, NVT], proj_dt, kind="ExternalInput")
    wT_d = nc.dram_tensor("wT", [128, KC, DIM], proj_dt, kind="ExternalInput")
    qso_d = nc.dram_tensor("qso", [128, BPT], sim_dt, kind="ExternalInput")
    onescol_d = nc.dram_tensor("onescol", [128, 1], sim_dt, kind="ExternalInput")
    onesrow_d = nc.dram_tensor("onesrow", [1, 128], sim_dt, kind="ExternalInput")
    ident_d = nc.dram_tensor("ident", [128, 128], sim_dt, kind="ExternalInput")
    out_d = nc.dram_tensor("out", [B, DPC], f32, kind="ExternalOutput")
    dbg = os.environ.get("KERNEL_DEBUG_OUT", "0") == "1"
    if dbg:
        dbgQT_d = nc.dram_tensor("dbg_QT", [128, TQ], sim_dt, kind="ExternalOutput")
        dbgD0_d = nc.dram_tensor("dbg_D0", [128, NM], sim_dt, kind="ExternalOutput")
        dbgDD_d = nc.dram_tensor("dbg_DD", [128, NM], sim_dt, kind="ExternalOutput")
        dbgMall_d = nc.dram_tensor("dbg_mall", [128, NTT, DPC], f32, kind="ExternalOutput")

    with tile.TileContext(nc) as tc, tc.tile_pool(name="persist", bufs=1) as per:
        # --- constants + persistent SBUF tensors ---------------------------
        wt = per.tile([128, KC, DIM], proj_dt, name="wt")
        qso = per.tile([128, BPT], sim_dt, name="qso")
        onescol = per.tile([128, 1], sim_dt, name="onescol")
        onesrow = per.tile([1, 128], sim_dt, name="onesrow")
        ident = per.tile([128, 128], sim_dt, name="ident")
        QT = per.tile([128, TQ], sim_dt, name="QT")       # q-proj [d, t] unnormalized
        DTn0 = per.tile([128, NM], sim_dt, name="DTn0")   # normalized doc half 0
        DTnH = per.tile([128, NM], sim_dt, name="DTnH")   # normalized doc half 1
        DTnD = per.tile([128, NM], sim_dt, name="DTnD")   # half1 - half0
        invnQ = per.tile([128, NTT], f32, name="invnQ")   # 1/|Q| per query token
        normQ = per.tile([128, NTT], f32, name="normQ")
        ssqD_row = per.tile([1, NVT], sim_dt, name="ssqD_row")
        bc = per.tile([128, NVT], f32, name="bc")         # broadcast 1/|D|
        outstage = per.tile([BPT, NTT * DPC], f32, name="outstage")
        if dbg:
            dbgM = per.tile([128, NTT, DPC], f32, name="dbgM")

        nc.sync.dma_start(wt[:], wT_d[:, :, :])
        nc.scalar.dma_start(ident[:], ident_d[:, :])
        nc.gpsimd.dma_start(qso[:], qso_d[:, :])
        nc.gpsimd.dma_start(onescol[:], onescol_d[:, :])
        nc.gpsimd.dma_start(onesrow[:], onesrow_d[:, :])

        # psq pool is opened before the D-phase pools and stays open across
        # the whole kernel: bank 0 is the Q-projection bank in both phases.
        psq_stack = tc.tile_pool(name="psqp", bufs=1, space="PSUM")
        psq_pool = psq_stack.__enter__()
        qs_stack = tc.tile_pool(name="qt_pool", bufs=1)
        qt_pool = qs_stack.__enter__()
        sqq_stack = tc.tile_pool(name="sqQ_pool", bufs=2)
        sqQ_pool = sqq_stack.__enter__()

        qts = {}

        def load_jg(jg):
            for k in range(KC):
                t_ = qt_pool.tile(
                    [128, QG], proj_dt, name=f"qt{k}_{jg}", tag=f"qt{k}",
                    bufs=3,
                )
                eng = nc.sync if k % 2 == 0 else nc.scalar
                eng.dma_start(t_[:], qT_d[k * 128:(k + 1) * 128,
                                          jg * QG:(jg + 1) * QG])
                qts[(k, jg)] = t_

        psq_live = {}

        def project_mm(j, ks):
            jg, r = divmod(j * 512, QG)
            if j not in psq_live:
                psq_live[j] = psq_pool.tile([128, 512], f32, name=f"psq{j}",
                                            tag="psq")
            psq = psq_live[j]
            for k in ks:
                nc.tensor.matmul(
                    psq[:, 0:512],
                    wt[:, k, :],
                    qts[(k, jg)][:, r:r + 512],
                    start=(k == 0),
                    stop=(k == KC - 1),
                )

        def project(j):
            sl = slice(j * 512, (j + 1) * 512)
            psq = psq_live.pop(j)
            nc.vector.tensor_copy(QT[:, sl], psq[:, 0:512])
            sq = sqQ_pool.tile([128, 512], sim_dt, name="sqq", tag="sqq")
            nc.scalar.activation(sq[:], psq[:, 0:512], AF.Square)
            # per-token |Q|^2 via ones-matmuls into spare psq columns (the
            # proj values there were already consumed by the two reads)
            for s in range(4):
                nc.tensor.matmul(
                    psq[:, 508 + s:509 + s],
                    sq[:, s * 128:(s + 1) * 128],
                    onescol[:],
                    start=True,
                    stop=True,
                )
            csl = slice(j * 4, (j + 1) * 4)
            nc.scalar.activation(normQ[:, csl], psq[:, 508:512], AF.Sqrt)
            nc.vector.reciprocal(invnQ[:, csl], normQ[:, csl])

        # PE warm-up: the HAM clock gate needs ~3.4us of sustained activity
        # to lift the PE from 1.2 to 2.4 GHz; burn the initial dT DMA wait
        # on dummy matmuls so the D projection starts warm.
        warm = psq_pool.tile([128, 128], f32, name="warm", tag="psq")
        for _ in range(20):
            nc.tensor.matmul(warm[:], ident[:], ident[:], start=True, stop=True)

        # ---------------- phase D: project + normalize doc tokens ----------
        with (
            tc.tile_pool(name="dt_pool", bufs=1) as dt_pool,
            tc.tile_pool(name="psD", bufs=1, space="PSUM") as psD,
            tc.tile_pool(name="ssD", bufs=1, space="PSUM") as ssD,
            tc.tile_pool(name="sqD_pool", bufs=2) as sqD_pool,
            tc.tile_pool(name="psB", bufs=1, space="PSUM") as psB,
        ):
            dts = []
            for k in range(KC):
                dtk = dt_pool.tile([128, NVT], proj_dt, name=f"dt{k}", tag=f"dt{k}")
                eng = nc.sync if k % 2 == 0 else nc.scalar
                eng.dma_start(dtk[:], dT_d[k * 128:(k + 1) * 128, :])
                dts.append(dtk)
            load_jg(0)
            load_jg(1)

            psd = psD.tile([128, NVT], f32, name="psd")
            for k in range(KC):
                for (off, ln) in d_chunks:
                    nc.tensor.matmul(
                        psd[:, off:off + ln],
                        wt[:, k, :],
                        dts[k][:, off:off + ln],
                        start=(k == 0),
                        stop=(k == KC - 1),
                    )
            for (off, ln) in d_chunks:
                sl = slice(off, off + ln)
                sq = sqD_pool.tile([128, 512], sim_dt, name="sqd", tag="sq")
                nc.scalar.activation(sq[:, :ln], psd[:, sl], AF.Square)
                ssd = ssD.tile([1, 512], f32, name="ssd", tag="ssd")
                nc.tensor.matmul(
                    ssd[:, :ln], onescol[:], sq[:, :ln], start=True, stop=True
                )
                nc.scalar.copy(ssqD_row[:, sl], ssd[:, :ln])

            # broadcast |D|^2 across partitions (bf16 ones-outer-product),
            # then full-width sqrt (ACT) + ~51-ULP reciprocal (DVE) per chunk
            for (off, ln) in d_chunks:
                sl = slice(off, off + ln)
                psb = psB.tile([128, 512], f32, name="psb", tag="psb")
                nc.tensor.matmul(
                    psb[:, :ln], onesrow[:], ssqD_row[:, sl], start=True, stop=True
                )
                rsq = sqD_pool.tile([128, 512], f32, name="rsq", tag="rsq")
                nc.scalar.activation(rsq[:, :ln], psb[:, :ln], AF.Sqrt)
                nc.vector.reciprocal_approx_fast(bc[:, sl], rsq[:, :ln])

            # normalize + split halves + delta (DVE; psd stays live here),
            # emitted per 8-doc half so group-0 sims can start after half 0
            psd_v = psd[:].rearrange("p (d v) -> p d v", v=NV)
            bc_v = bc[:].rearrange("p (d v) -> p d v", v=NV)
            D0_v = DTn0[:].rearrange("p (d v) -> p d v", v=NV2)
            DH_v = DTnH[:].rearrange("p (d v) -> p d v", v=NV2)
            for h in range(2):
                ds = slice(h * GD, (h + 1) * GD)
                fs = slice(h * GW, (h + 1) * GW)
                nc.vector.tensor_tensor(
                    D0_v[:, ds], psd_v[:, ds, 0:NV2], bc_v[:, ds, 0:NV2],
                    op=ALU.mult,
                )
                nc.vector.tensor_tensor(
                    DH_v[:, ds], psd_v[:, ds, NV2:NV], bc_v[:, ds, NV2:NV],
                    op=ALU.mult,
                )
                nc.vector.tensor_tensor(
                    DTnD[:, fs], DTnH[:, fs], DTn0[:, fs], op=ALU.subtract
                )

            # early Q chunks overlap the D-phase tail
            project_mm(0, range(KC))
            project(0)
            project_mm(1, range(KC))
            project(1)
            load_jg(2)

        # ---------- phase S: sim groups with LIN1 fold ---------------------
        # One merged PSUM tile per group holds S0/M in cols [0, GW) and the
        # delta sims in [GW, 2*GW): 3 banks, double-buffered, so consecutive
        # groups pipeline without a separate (bank-starved) delta pool.
        # Delta matmuls run FIRST: their start=True clears the has_written
        # bits of the whole bank, which must precede the M matmul that
        # shares the middle bank (the identity matmul then accumulates
        # onto M's bits only).
        dz_chunks = []
        off = GW
        while off < 2 * GW:
            nxt = min((off // 512 + 1) * 512, 2 * GW)
            dz_chunks.append((off, nxt - off))
            off = nxt

        with (
            tc.tile_pool(name="psS", bufs=2, space="PSUM") as psS,
            tc.tile_pool(name="psO", bufs=1, space="PSUM") as psO,
            tc.tile_pool(name="r_pool", bufs=2) as r_pool,
            tc.tile_pool(name="m_pool", bufs=2) as m_pool,
        ):
            psout = psO.tile([BPT, NTT * DPC], f32, name="psout")

            def simgroup(t, g, mall, weave=None):
                lq = QT[:, t * 128:(t + 1) * 128]
                d0 = g * GW
                ps = psS.tile([128, 2 * GW], f32, name="ps", tag="ps")
                for (off, ln) in dz_chunks:
                    nc.tensor.matmul(
                        ps[:, off:off + ln],
                        lq, DTnD[:, d0 + off - GW:d0 + off - GW + ln],
                        start=True, stop=True,
                    )
                for (off, ln) in reversed(g_chunks):
                    nc.tensor.matmul(
                        ps[:, off:off + ln], lq, DTn0[:, d0 + off:d0 + off + ln],
                        start=True, stop=False,
                    )
                r = r_pool.tile([128, GW], sim_dt, name="r", tag="r")
                nc.scalar.activation(r[:], ps[:, GW:2 * GW], AF.Relu)
                if weave:
                    weave()
                for (off, ln) in g_chunks:
                    nc.tensor.matmul(
                        ps[:, off:off + ln], ident[:], r[:, off:off + ln],
                        start=False, stop=(off + ln == GW),
                    )
                nc.vector.reduce_max(
                    mall[:, g * GD:(g + 1) * GD],
                    ps[:, 0:GW].rearrange("p (d v) -> p d v", v=NV2),
                    axis=AX.X,
                )

            mals_q = []

            def flush_psout():
                t_, mals_ = mals_q.pop(0)
                nc.tensor.matmul(
                    psout[:, t_ * DPC:(t_ + 1) * DPC],
                    qso[:],
                    mals_[:],
                    start=True,
                    stop=True,
                )

            def simtile(t, weave=None):
                # the psout matmul of tile t-2 goes first: its gpsimd-scaled
                # rhs is ready by now, so it never head-of-line-blocks the
                # strict-FIFO PE queue the way an eagerly-issued one would
                if len(mals_q) >= 2:
                    flush_psout()
                mall = m_pool.tile([128, DPC], f32, name="mall", tag="mall",
                                   bufs=4)
                for g in range(NG):
                    simgroup(t, g, mall, weave)
                # fold 1/|Q| into the maxes on the (otherwise idle) gpsimd,
                # so the final sum matmul uses the constant block-ones qso
                mals = m_pool.tile([128, DPC], sim_dt, name="mals", tag="mals",
                                   bufs=4)
                nc.gpsimd.tensor_scalar(
                    mals[:], mall[:], invnQ[:, t:t + 1], None, op0=ALU.mult
                )
                if dbg:
                    nc.vector.tensor_copy(dbgM[:, t, :], mals[:])
                mals_q.append((t, mals))

            for j in range(NQCH):
                kstep = iter(range(KC))

                def weave(_j=j, _ks=kstep):
                    if _j + 2 < NQCH:
                        k = next(_ks, None)
                        if k is not None:
                            project_mm(_j + 2, [k])

                if j == 1:
                    load_jg(3)
                for ti, t in enumerate(range(j * 4, (j + 1) * 4)):
                    simtile(t, weave=weave)
                    if j + 2 < NQCH and ti == 2:
                        project(j + 2)
            while mals_q:
                flush_psout()
            if dbg:
                nc.sync.dma_start(dbgQT_d[:, :], QT[:])
                nc.sync.dma_start(dbgD0_d[:, :], DTn0[:])
                nc.sync.dma_start(dbgDD_d[:, :], DTnD[:])
                nc.sync.dma_start(dbgMall_d[:, :, :], dbgM[:])
            nc.scalar.copy(outstage[:], psout[:])
            nc.sync.dma_start(
                out_d[:, :].rearrange("(t f) c -> f t c", f=BPT),
                outstage[:].rearrange("f (t c) -> f t c", c=DPC),
            )
        sqq_stack.__exit__(None, None, None)
        qs_stack.__exit__(None, None, None)
        psq_stack.__exit__(None, None, None)

    nc.compile()
    return nc


def _host_prep(q_hidden, d_hidden, W, d_mask):
    import ml_dtypes

    q = np.ascontiguousarray(np.asarray(q_hidden, dtype=np.float32))
    d = np.ascontiguousarray(np.asarray(d_hidden, dtype=np.float32))
    w = np.ascontiguousarray(np.asarray(W, dtype=np.float32))
    mask = np.asarray(d_mask, dtype=bool)

    nv = mask.sum(axis=1)
    NV = int(-(-max(int(nv.max()), 16) // 8) * 8)
    NV = min(NV, ((LD + 7) // 8) * 8)

    # per-doc gather indices: valid tokens first, padded with the first
    # valid token (duplicates never change a max)
    idx = np.zeros((B, NV), dtype=np.intp)
    for c in range(B):
        v = np.flatnonzero(mask[c])
        row = np.full(NV, v[0], dtype=np.intp)
        row[:min(len(v), NV)] = v[:NV]
        idx[c] = row

    dG = d[np.arange(B)[:, None], idx, :]          # [B, NV, HID]

    # activations/weights are shipped bf16 so HBM traffic halves
    proj_np = ml_dtypes.bfloat16 if SIM_MODE == "bf16" else np.float32
    qT = np.ascontiguousarray(q.reshape(TQ, HID).T.astype(proj_np))  # [HID, TQ]
    # W.T rearranged so the [128, KC, DIM] SBUF tile is one contiguous DMA:
    # wTp[p, k, d] = W[d, k*128+p]
    wT = np.ascontiguousarray(
        w.T.reshape(KC, 128, DIM).transpose(1, 0, 2).astype(proj_np)
    )
    dT_cores = []
    for m in range(NCORES):
        blk = dG[m * DPC:(m + 1) * DPC].reshape(DPC * NV, HID)
        dT_cores.append(np.ascontiguousarray(blk.T.astype(proj_np)))

    qso = np.zeros((128, 128 // LQ), dtype=ml_dtypes.bfloat16 if SIM_MODE == "bf16" else np.float32)
    for p in range(128):
        qso[p, p // LQ] = 1.0
    ones_dt = ml_dtypes.bfloat16 if SIM_MODE == "bf16" else np.float32
    onescol = np.ones((128, 1), dtype=ones_dt)
    onesrow = np.ones((1, 128), dtype=ones_dt)
    ident = np.eye(128, dtype=ones_dt)
    return NV, qT, wT, dT_cores, qso, onescol, onesrow, ident


def kernel(q_hidden, d_hidden, W, d_mask):
    from concourse.bass_utils import run_bass_kernel_spmd

    NV, qT, wT, dT_cores, qso, onescol, onesrow, ident = _host_prep(
        q_hidden, d_hidden, W, d_mask
    )
    nc = _build_program(NV)

    in_maps = [
        {
            "qT": qT,
            "dT": dT_cores[m],
            "wT": wT,
            "qso": qso,
            "onescol": onescol,
            "onesrow": onesrow,
            "ident": ident,
        }
        for m in range(NCORES)
    ]
    res = run_bass_kernel_spmd(nc, in_maps, core_ids=list(range(NCORES)))
    out = np.concatenate(
        [res.results[m]["out"] for m in range(NCORES)], axis=1
    )
    return np.ascontiguousarray(out.astype(np.float32))


# revision 18
# speedup vs baseline: 1.3529x; 1.0256x over previous
"""ColBERT intra-batch MaxSim scoring kernel for 8 Trainium2 NeuronCores.

Math (see reference):
  Q = l2norm(q_hidden @ W.T)                       [B, LQ, DIM]
  D = l2norm(d_hidden @ W.T); D masked             [B, LD, DIM]
  sim[b,c,q,k] = Q[b,q]·D[c,k]; masked k -> -inf
  out[b,c] = sum_q max_k sim

Sharding: docs (dim c) are sharded 16-per-core; q_hidden/W replicated.
Each core computes its [B, 16] slice of the score matrix.

Device-side structure (v2 — "LIN1" max-split):
  * Host pre-transposes activations to [HID, tokens] (bf16 to halve HBM
    traffic) and folds the doc mask away by gathering valid tokens to the
    front, padding the tail with copies of the doc's first valid token
    (duplicates never change a max).
  * The max over each doc's NV tokens is split one level in the *linear*
    domain: with per-doc token halves h0/h1 and the normalized doc
    embeddings D0/D1, max(s0,s1) = s0 + relu(s1-s0), and s1-s0 is itself
    a matmul against Ddelta = D1-D0.  So per 8-doc group the device runs
      PE:  S0 = Q·D0 (PSUM, group open)     SΔ = Q·Ddelta (PSUM)
      ACT: r = relu(SΔ) -> SBUF bf16
      PE:  S0 += I·r  (identity matmul accumulates into the open group)
      DVE: reduce_max over NV/2-wide doc slots
    which halves the DVE tensor_reduce (the v1 bottleneck at ~90us busy)
    at the cost of ~1.5x PE rows and the ACT relu, balancing all three
    engines.
  * Q is NOT normalized before the sim matmuls: max_k is invariant under
    a positive per-query scale, so 1/|Q| is folded into the block-ones
    lhsT of the final query-sum matmul.  Per-token |Q|^2 is computed by
    ones-matmuls into spare columns of the Q-projection PSUM bank.
  * D norms: ones-matmul sumsq row, sqrt (ACT) + fast reciprocal (DVE),
    broadcast via K=1 ones outer-product, applied by the DVE while
    splitting D into D0 / D1 / Ddelta.
  * PSUM budget (8 banks): psq 1 | S0 double-buffered 2x2 | SΔ 2 |
    psout 1.
"""

import os

import numpy as np

B, LQ, LD, HID, DIM = 128, 32, 256, 768, 128
NCORES = 8
DPC = B // NCORES          # docs per core
TQ = B * LQ                # total query tokens
KC = HID // 128            # contraction chunks for the projection
GD = 8                     # docs per sim group (2 groups per 128-token tile)

SIM_MODE = os.environ.get("KERNEL_SIM_MODE", "bf16")


def _chunks(total, step):
    """[(off, len)] cut at `step` boundaries — a matmul's PSUM output must
    stay inside a single 512-float bank, so chunks may never straddle one."""
    return [(o, min(step, total - o)) for o in range(0, total, step)]


def _build_program(NV):
    import concourse.bass as bass  # noqa: F401
    import concourse.tile as tile
    from concourse import bacc, mybir

    f32 = mybir.dt.float32
    AF = mybir.ActivationFunctionType
    AX = mybir.AxisListType
    ALU = mybir.AluOpType

    sim_dt = {"bf16": mybir.dt.bfloat16, "f32": f32}[SIM_MODE]
    proj_dt = sim_dt

    NV2 = NV // 2           # compacted tokens per doc half
    NVT = DPC * NV          # compacted doc tokens per core
    NM = DPC * NV2          # columns after the LIN1 fold (also = |Ddelta|)
    GW = GD * NV2           # sim-group width (8 docs)
    NG = DPC // GD          # sim groups per tile
    NQCH = TQ // 512        # q-projection column chunks
    NTT = TQ // 128         # sim lhsT tiles (query-token tiles)
    BPT = 128 // LQ         # batch entries per query-token tile
    QG = 1024               # qt DMA column-group width
    d_chunks = _chunks(NVT, 512)   # d-projection column chunks
    g_chunks = _chunks(GW, 512)    # sim-group matmul N chunks (512 + 96)

    nc = bacc.Bacc(
        "TRN2",
        target_bir_lowering=False,
        debug=False,
        num_devices=NCORES,
    )

    qT_d = nc.dram_tensor("qT", [HID, TQ], proj_dt, kind="ExternalInput")
    dT_d = nc.dram_tensor("dT", [HID, NVT], proj_dt, kind="ExternalInput")
    wT_d = nc.dram_tensor("wT", [128, KC, DIM], proj_dt, kind="ExternalInput")
    qso_d = nc.dram_tensor("qso", [128, BPT], sim_dt, kind="ExternalInput")
    onescol_d = nc.dram_tensor("onescol", [128, 1], sim_dt, kind="ExternalInput")
    onesrow_d = nc.dram_tensor("onesrow", [1, 128], sim_dt, kind="ExternalInput")
    ident_d = nc.dram_tensor("ident", [128, 128], sim_dt, kind="ExternalInput")
    out_d = nc.dram_tensor("out", [B, DPC], f32, kind="ExternalOutput")
    dbg = os.environ.get("KERNEL_DEBUG_OUT", "0") == "1"
    if dbg:
        dbgQT_d = nc.dram_tensor("dbg_QT", [128, TQ], sim_dt, kind="ExternalOutput")
        dbgD0_d = nc.dram_tensor("dbg_D0", [128, NM], sim_dt, kind="ExternalOutput")
        dbgDD_d = nc.dram_tensor("dbg_DD", [128, NM], sim_dt, kind="ExternalOutput")
        dbgMall_d = nc.dram_tensor("dbg_mall", [128, NTT, DPC], f32, kind="ExternalOutput")

    with tile.TileContext(nc) as tc, tc.tile_pool(name="persist", bufs=1) as per:
        # --- constants + persistent SBUF tensors ---------------------------
        wt = per.tile([128, KC, DIM], proj_dt, name="wt")
        qso = per.tile([128, BPT], sim_dt, name="qso")
        onescol = per.tile([128, 1], sim_dt, name="onescol")
        onesrow = per.tile([1, 128], sim_dt, name="onesrow")
        ident = per.tile([128, 128], sim_dt, name="ident")
        QT = per.tile([128, TQ], sim_dt, name="QT")       # q-proj [d, t] unnormalized
        DTn0 = per.tile([128, NM], sim_dt, name="DTn0")   # normalized doc half 0
        DTnH = per.tile([128, NM], sim_dt, name="DTnH")   # normalized doc half 1
        DTnD = per.tile([128, NM], sim_dt, name="DTnD")   # half1 - half0
        invnQ = per.tile([128, NTT], f32, name="invnQ")   # 1/|Q| per query token
        normQ = per.tile([128, NTT], f32, name="normQ")
        ssqD_row = per.tile([1, NVT], sim_dt, name="ssqD_row")
        bc = per.tile([128, NVT], f32, name="bc")         # broadcast 1/|D|
        outstage = per.tile([BPT, NTT * DPC], f32, name="outstage")
        if dbg:
            dbgM = per.tile([128, NTT, DPC], f32, name="dbgM")

        nc.sync.dma_start(wt[:], wT_d[:, :, :])
        nc.scalar.dma_start(ident[:], ident_d[:, :])
        nc.gpsimd.dma_start(qso[:], qso_d[:, :])
        nc.gpsimd.dma_start(onescol[:], onescol_d[:, :])
        nc.gpsimd.dma_start(onesrow[:], onesrow_d[:, :])

        # psq pool is opened before the D-phase pools and stays open across
        # the whole kernel: bank 0 is the Q-projection bank in both phases.
        psq_stack = tc.tile_pool(name="psqp", bufs=1, space="PSUM")
        psq_pool = psq_stack.__enter__()
        qs_stack = tc.tile_pool(name="qt_pool", bufs=1)
        qt_pool = qs_stack.__enter__()
        sqq_stack = tc.tile_pool(name="sqQ_pool", bufs=2)
        sqQ_pool = sqq_stack.__enter__()

        qts = {}

        def load_jg(jg):
            for k in range(KC):
                t_ = qt_pool.tile(
                    [128, QG], proj_dt, name=f"qt{k}_{jg}", tag=f"qt{k}",
                    bufs=3,
                )
                # late groups go entirely on the sync queue: its stream is
                # drained by ~50us, while the scalar-triggered stream was
                # observed still delivering at 147us — starving chunks 6-7
                eng = nc.sync if (jg >= 2 or k % 2 == 0) else nc.scalar
                eng.dma_start(t_[:], qT_d[k * 128:(k + 1) * 128,
                                          jg * QG:(jg + 1) * QG])
                qts[(k, jg)] = t_

        psq_live = {}

        def project_mm(j, ks):
            jg, r = divmod(j * 512, QG)
            if j not in psq_live:
                psq_live[j] = psq_pool.tile([128, 512], f32, name=f"psq{j}",
                                            tag="psq")
            psq = psq_live[j]
            for k in ks:
                nc.tensor.matmul(
                    psq[:, 0:512],
                    wt[:, k, :],
                    qts[(k, jg)][:, r:r + 512],
                    start=(k == 0),
                    stop=(k == KC - 1),
                )

        def project(j):
            sl = slice(j * 512, (j + 1) * 512)
            psq = psq_live.pop(j)
            nc.vector.tensor_copy(QT[:, sl], psq[:, 0:512])
            sq = sqQ_pool.tile([128, 512], sim_dt, name="sqq", tag="sqq")
            nc.scalar.activation(sq[:], psq[:, 0:512], AF.Square)
            # per-token |Q|^2 via ones-matmuls into spare psq columns (the
            # proj values there were already consumed by the two reads)
            for s in range(4):
                nc.tensor.matmul(
                    psq[:, 508 + s:509 + s],
                    sq[:, s * 128:(s + 1) * 128],
                    onescol[:],
                    start=True,
                    stop=True,
                )
            csl = slice(j * 4, (j + 1) * 4)
            nc.scalar.activation(normQ[:, csl], psq[:, 508:512], AF.Sqrt)
            nc.vector.reciprocal(invnQ[:, csl], normQ[:, csl])

        # PE warm-up: the HAM clock gate needs ~3.4us of sustained activity
        # to lift the PE from 1.2 to 2.4 GHz; burn the initial dT DMA wait
        # on dummy matmuls so the D projection starts warm.
        warm = psq_pool.tile([128, 128], f32, name="warm", tag="psq")
        for _ in range(20):
            nc.tensor.matmul(warm[:], ident[:], ident[:], start=True, stop=True)

        # ---------------- phase D: project + normalize doc tokens ----------
        with (
            tc.tile_pool(name="dt_pool", bufs=1) as dt_pool,
            tc.tile_pool(name="psD", bufs=1, space="PSUM") as psD,
            tc.tile_pool(name="ssD", bufs=1, space="PSUM") as ssD,
            tc.tile_pool(name="sqD_pool", bufs=2) as sqD_pool,
            tc.tile_pool(name="psB", bufs=1, space="PSUM") as psB,
        ):
            dts = []
            for k in range(KC):
                dtk = dt_pool.tile([128, NVT], proj_dt, name=f"dt{k}", tag=f"dt{k}")
                eng = nc.sync if k % 2 == 0 else nc.scalar
                eng.dma_start(dtk[:], dT_d[k * 128:(k + 1) * 128, :])
                dts.append(dtk)
            load_jg(0)
            load_jg(1)

            psd = psD.tile([128, NVT], f32, name="psd")
            for k in range(KC):
                for (off, ln) in d_chunks:
                    nc.tensor.matmul(
                        psd[:, off:off + ln],
                        wt[:, k, :],
                        dts[k][:, off:off + ln],
                        start=(k == 0),
                        stop=(k == KC - 1),
                    )
            for (off, ln) in d_chunks:
                sl = slice(off, off + ln)
                sq = sqD_pool.tile([128, 512], sim_dt, name="sqd", tag="sq")
                nc.scalar.activation(sq[:, :ln], psd[:, sl], AF.Square)
                ssd = ssD.tile([1, 512], f32, name="ssd", tag="ssd")
                nc.tensor.matmul(
                    ssd[:, :ln], onescol[:], sq[:, :ln], start=True, stop=True
                )
                nc.scalar.copy(ssqD_row[:, sl], ssd[:, :ln])

            # broadcast |D|^2 across partitions (bf16 ones-outer-product),
            # then full-width sqrt (ACT) + ~51-ULP reciprocal (DVE) per chunk
            for (off, ln) in d_chunks:
                sl = slice(off, off + ln)
                psb = psB.tile([128, 512], f32, name="psb", tag="psb")
                nc.tensor.matmul(
                    psb[:, :ln], onesrow[:], ssqD_row[:, sl], start=True, stop=True
                )
                rsq = sqD_pool.tile([128, 512], f32, name="rsq", tag="rsq")
                nc.scalar.activation(rsq[:, :ln], psb[:, :ln], AF.Sqrt)
                nc.vector.reciprocal_approx_fast(bc[:, sl], rsq[:, :ln])

            # normalize + split halves + delta (DVE; psd stays live here),
            # emitted per 8-doc half so group-0 sims can start after half 0
            psd_v = psd[:].rearrange("p (d v) -> p d v", v=NV)
            bc_v = bc[:].rearrange("p (d v) -> p d v", v=NV)
            D0_v = DTn0[:].rearrange("p (d v) -> p d v", v=NV2)
            DH_v = DTnH[:].rearrange("p (d v) -> p d v", v=NV2)
            for h in range(2):
                ds = slice(h * GD, (h + 1) * GD)
                fs = slice(h * GW, (h + 1) * GW)
                nc.vector.tensor_tensor(
                    D0_v[:, ds], psd_v[:, ds, 0:NV2], bc_v[:, ds, 0:NV2],
                    op=ALU.mult,
                )
                nc.vector.tensor_tensor(
                    DH_v[:, ds], psd_v[:, ds, NV2:NV], bc_v[:, ds, NV2:NV],
                    op=ALU.mult,
                )
                nc.vector.tensor_tensor(
                    DTnD[:, fs], DTnH[:, fs], DTn0[:, fs], op=ALU.subtract
                )

            # early Q chunks overlap the D-phase tail
            project_mm(0, range(KC))
            project(0)
            project_mm(1, range(KC))
            project(1)
            load_jg(2)

        # ---------- phase S: sim groups with LIN1 fold ---------------------
        # One merged PSUM tile per group holds S0/M in cols [0, GW) and the
        # delta sims in [GW, 2*GW): 3 banks, double-buffered, so consecutive
        # groups pipeline without a separate (bank-starved) delta pool.
        # Delta matmuls run FIRST: their start=True clears the has_written
        # bits of the whole bank, which must precede the M matmul that
        # shares the middle bank (the identity matmul then accumulates
        # onto M's bits only).
        dz_chunks = []
        off = GW
        while off < 2 * GW:
            nxt = min((off // 512 + 1) * 512, 2 * GW)
            dz_chunks.append((off, nxt - off))
            off = nxt

        with (
            tc.tile_pool(name="psS", bufs=2, space="PSUM") as psS,
            tc.tile_pool(name="psO", bufs=1, space="PSUM") as psO,
            tc.tile_pool(name="r_pool", bufs=2) as r_pool,
            tc.tile_pool(name="m_pool", bufs=2) as m_pool,
        ):
            psout = psO.tile([BPT, NTT * DPC], f32, name="psout")

            def simgroup(t, g, mall, weave=None):
                lq = QT[:, t * 128:(t + 1) * 128]
                d0 = g * GW
                ps = psS.tile([128, 2 * GW], f32, name="ps", tag="ps")
                for (off, ln) in dz_chunks:
                    nc.tensor.matmul(
                        ps[:, off:off + ln],
                        lq, DTnD[:, d0 + off - GW:d0 + off - GW + ln],
                        start=True, stop=True,
                    )
                for (off, ln) in reversed(g_chunks):
                    nc.tensor.matmul(
                        ps[:, off:off + ln], lq, DTn0[:, d0 + off:d0 + off + ln],
                        start=True, stop=False,
                    )
                r = r_pool.tile([128, GW], sim_dt, name="r", tag="r")
                nc.scalar.activation(r[:], ps[:, GW:2 * GW], AF.Relu)
                if weave:
                    weave()
                for (off, ln) in g_chunks:
                    nc.tensor.matmul(
                        ps[:, off:off + ln], ident[:], r[:, off:off + ln],
                        start=False, stop=(off + ln == GW),
                    )
                nc.vector.reduce_max(
                    mall[:, g * GD:(g + 1) * GD],
                    ps[:, 0:GW].rearrange("p (d v) -> p d v", v=NV2),
                    axis=AX.X,
                )

            mals_q = []

            def flush_psout():
                t_, mals_ = mals_q.pop(0)
                nc.tensor.matmul(
                    psout[:, t_ * DPC:(t_ + 1) * DPC],
                    qso[:],
                    mals_[:],
                    start=True,
                    stop=True,
                )

            def simtile(t, weave=None):
                # the psout matmul of tile t-2 goes first: its gpsimd-scaled
                # rhs is ready by now, so it never head-of-line-blocks the
                # strict-FIFO PE queue the way an eagerly-issued one would
                if len(mals_q) >= 2:
                    flush_psout()
                mall = m_pool.tile([128, DPC], f32, name="mall", tag="mall",
                                   bufs=4)
                for g in range(NG):
                    simgroup(t, g, mall, weave)
                # fold 1/|Q| into the maxes on the (otherwise idle) gpsimd,
                # so the final sum matmul uses the constant block-ones qso
                mals = m_pool.tile([128, DPC], sim_dt, name="mals", tag="mals",
                                   bufs=4)
                nc.gpsimd.tensor_scalar(
                    mals[:], mall[:], invnQ[:, t:t + 1], None, op0=ALU.mult
                )
                if dbg:
                    nc.vector.tensor_copy(dbgM[:, t, :], mals[:])
                mals_q.append((t, mals))

            for j in range(NQCH):
                kstep = iter(range(KC))

                def weave(_j=j, _ks=kstep):
                    if _j + 2 < NQCH:
                        k = next(_ks, None)
                        if k is not None:
                            project_mm(_j + 2, [k])

                if j == 1:
                    load_jg(3)
                for ti, t in enumerate(range(j * 4, (j + 1) * 4)):
                    simtile(t, weave=weave)
                    if j + 2 < NQCH and ti == 2:
                        project(j + 2)
            while mals_q:
                flush_psout()
            if dbg:
                nc.sync.dma_start(dbgQT_d[:, :], QT[:])
                nc.sync.dma_start(dbgD0_d[:, :], DTn0[:])
                nc.sync.dma_start(dbgDD_d[:, :], DTnD[:])
                nc.sync.dma_start(dbgMall_d[:, :, :], dbgM[:])
            nc.scalar.copy(outstage[:], psout[:])
            nc.sync.dma_start(
                out_d[:, :].rearrange("(t f) c -> f t c", f=BPT),
                outstage[:].rearrange("f (t c) -> f t c", c=DPC),
            )
        sqq_stack.__exit__(None, None, None)
        qs_stack.__exit__(None, None, None)
        psq_stack.__exit__(None, None, None)

    nc.compile()
    return nc


def _host_prep(q_hidden, d_hidden, W, d_mask):
    import ml_dtypes

    q = np.ascontiguousarray(np.asarray(q_hidden, dtype=np.float32))
    d = np.ascontiguousarray(np.asarray(d_hidden, dtype=np.float32))
    w = np.ascontiguousarray(np.asarray(W, dtype=np.float32))
    mask = np.asarray(d_mask, dtype=bool)

    nv = mask.sum(axis=1)
    NV = int(-(-max(int(nv.max()), 16) // 8) * 8)
    NV = min(NV, ((LD + 7) // 8) * 8)

    # per-doc gather indices: valid tokens first, padded with the first
    # valid token (duplicates never change a max)
    idx = np.zeros((B, NV), dtype=np.intp)
    for c in range(B):
        v = np.flatnonzero(mask[c])
        row = np.full(NV, v[0], dtype=np.intp)
        row[:min(len(v), NV)] = v[:NV]
        idx[c] = row

    dG = d[np.arange(B)[:, None], idx, :]          # [B, NV, HID]

    # activations/weights are shipped bf16 so HBM traffic halves
    proj_np = ml_dtypes.bfloat16 if SIM_MODE == "bf16" else np.float32
    qT = np.ascontiguousarray(q.reshape(TQ, HID).T.astype(proj_np))  # [HID, TQ]
    # W.T rearranged so the [128, KC, DIM] SBUF tile is one contiguous DMA:
    # wTp[p, k, d] = W[d, k*128+p]
    wT = np.ascontiguousarray(
        w.T.reshape(KC, 128, DIM).transpose(1, 0, 2).astype(proj_np)
    )
    dT_cores = []
    for m in range(NCORES):
        blk = dG[m * DPC:(m + 1) * DPC].reshape(DPC * NV, HID)
        dT_cores.append(np.ascontiguousarray(blk.T.astype(proj_np)))

    qso = np.zeros((128, 128 // LQ), dtype=ml_dtypes.bfloat16 if SIM_MODE == "bf16" else np.float32)
    for p in range(128):
        qso[p, p // LQ] = 1.0
    ones_dt = ml_dtypes.bfloat16 if SIM_MODE == "bf16" else np.float32
    onescol = np.ones((128, 1), dtype=ones_dt)
    onesrow = np.ones((1, 128), dtype=ones_dt)
    ident = np.eye(128, dtype=ones_dt)
    return NV, qT, wT, dT_cores, qso, onescol, onesrow, ident


def kernel(q_hidden, d_hidden, W, d_mask):
    from concourse.bass_utils import run_bass_kernel_spmd

    NV, qT, wT, dT_cores, qso, onescol, onesrow, ident = _host_prep(
        q_hidden, d_hidden, W, d_mask
    )
    nc = _build_program(NV)

    in_maps = [
        {
            "qT": qT,
            "dT": dT_cores[m],
            "wT": wT,
            "qso": qso,
            "onescol": onescol,
            "onesrow": onesrow,
            "ident": ident,
        }
        for m in range(NCORES)
    ]
    res = run_bass_kernel_spmd(nc, in_maps, core_ids=list(range(NCORES)))
    out = np.concatenate(
        [res.results[m]["out"] for m in range(NCORES)], axis=1
    )
    return np.ascontiguousarray(out.astype(np.float32))
